# revision 23
# baseline (speedup 1.0000x reference)
"""Trainium2 Bass kernel for the NeuralSheet settling network (S=64).

Math (see reference):
    afferent = A @ x                      A: (4096, 4096), x: 4096
    lri  = corr * mask, row-normalized (+eps)
    W    = exc - lri                      (4096, 4096)
    r_0  = 0
    r_t  = tanh(relu(afferent + 2*(W @ r_{t-1}) - thr))   x 20
    out  = r_20

Distribution: the 4096 output units are row-sharded 512 per core across 8
NeuronCores.  Each core:
  - streams its 512-row slices of A / exc / corr / mask from HBM once,
  - builds 2*W_shard in SBUF and PE-transposes it into column-major tiles
    WT[k][p, i] = 2*W[row i, col 32*p + k]  (k = 0..31 chunk, p = partition),
  - computes afferent_shard with fused multiply-reduce on the vector engine,
  - runs the 20 settling iterations: matvec on the tensor engine with the
    response chunk as the (tiny) stationary operand and WT streaming, then
    tanh/relu, then AllGathers the 512-long response shard (19 collectives).

Precision: the settling dynamics amplify per-step rounding ~1.5e3x, so W and
r stay at (near-)fp32.  MODE selects the matvec path:
  "f32r"   - single chain of float32r matmuls (1 cyc/row on TRN2)
  "bf16x3" - W and r each split hi+lo in bf16; 3 accumulation chains
"""

import numpy as np

import concourse.bacc as bacc
import concourse.bass as bass
import concourse.mybir as mybir
import concourse.tile as tile
from concourse.bass_utils import run_bass_kernel_spmd

S2 = 4096                  # sheet units (S*S) == input pixels (I*I)
NCORES = 8
RP = S2 // NCORES          # 512 rows per core
RT = RP // 128             # 4 row tiles of 128 partitions
KC = S2 // 128             # 32 contraction chunks
HALF = S2 // 2
ITERS = 20
EPS = 1e-11

F32 = mybir.dt.float32
F32R = mybir.dt.float32r
BF16 = mybir.dt.bfloat16
OP = mybir.AluOpType
AF = mybir.ActivationFunctionType

MODE = "bf16x3"


def build(mode=MODE, use_ag=True, iters=ITERS, do_w=True, do_tr=True):
    nc = bacc.Bacc(
        "TRN2", target_bir_lowering=False, debug=False, num_devices=NCORES
    )

    a_d = nc.dram_tensor("a_shard", [RP, S2], F32, kind="ExternalInput")
    e_d = nc.dram_tensor("e_shard", [RP, S2], F32, kind="ExternalInput")
    c_d = nc.dram_tensor("c_shard", [RP, S2], F32, kind="ExternalInput")
    m_d = nc.dram_tensor("m_shard", [RP, S2], F32, kind="ExternalInput")
    xb_d = nc.dram_tensor("x_bcast", [128, S2], F32, kind="ExternalInput")
    thr_d = nc.dram_tensor("thr_shard", [1, RP], F32, kind="ExternalInput")
    id_d = nc.dram_tensor("ident", [128, 128], F32, kind="ExternalInput")
    out_d = nc.dram_tensor("r_out", [1, RP], F32, kind="ExternalOutput")
    dbg_d = nc.dram_tensor("dbg", [1, RP], F32, kind="ExternalOutput")

    groups = [list(range(NCORES))]

    with tile.TileContext(nc) as tc:
        with (
            tc.tile_pool(name="sb", bufs=1) as sb,
            tc.tile_pool(name="ps", bufs=1, space="PSUM") as psp,
            tc.tile_pool(name="dr", bufs=1, space="DRAM") as dr,
        ):
            ident = sb.tile([128, 128], F32)
            nc.sync.dma_start(out=ident[:], in_=id_d[:])
            xb = sb.tile([128, S2], F32)
            nc.sync.dma_start(out=xb[:], in_=xb_d[:])

            # lateral weight storage, column-major chunks
            if mode == "f32r":
                wt_h = sb.tile([128, KC, RP], F32, name="wt_h")
                wt_l = None
            else:
                wt_h = sb.tile([128, KC, RP], BF16, name="wt_h")
                wt_l = sb.tile([128, KC, RP], BF16, name="wt_l")

            # ---- afferent: aff_col[p, t] = sum_j A[128t+p, j] * x[j] ----
            aff_col = sb.tile([128, RT], F32)
            for t in range(RT):
                aff2 = sb.tile([128, 2], F32, tag="aff2", bufs=2, name="aff2")
                for h in range(2):
                    a_t = sb.tile(
                        [128, HALF], F32, tag="h2048", bufs=2, name="a_t"
                    )
                    nc.sync.dma_start(
                        out=a_t[:],
                        in_=a_d[128 * t : 128 * (t + 1), HALF * h : HALF * (h + 1)],
                    )
                    prod_a = sb.tile(
                        [128, S2], F32, tag="big16", bufs=1, name="prod_a"
                    )
                    nc.vector.tensor_tensor(
                        out=prod_a[:, :HALF],
                        in0=a_t[:],
                        in1=xb[:, HALF * h : HALF * (h + 1)],
                        op=OP.mult,
                    )
                    nc.scalar.activation(
                        out=a_t[:],
                        in_=prod_a[:, :HALF],
                        func=AF.Identity,
                        accum_out=aff2[:, h : h + 1],
                    )
                nc.vector.tensor_tensor(
                    out=aff_col[:, t : t + 1],
                    in0=aff2[:, 0:1],
                    in1=aff2[:, 1:2],
                    op=OP.add,
                )

            # ---- build 2*W rows, transpose into WT ----
            for t in range(RT if do_w else 0):
                tprod = sb.tile([128, S2], F32, tag="big16", bufs=1, name="tprod")
                rs2 = sb.tile([128, 2], F32, tag="rs2", bufs=2, name="rs2")
                for h in range(2):
                    c_t = sb.tile([128, HALF], F32, tag="c_t", bufs=2, name="c_t")
                    m_t = sb.tile([128, HALF], F32, tag="m_t", bufs=2, name="m_t")
                    nc.sync.dma_start(
                        out=c_t[:],
                        in_=c_d[128 * t : 128 * (t + 1), HALF * h : HALF * (h + 1)],
                    )
                    nc.sync.dma_start(
                        out=m_t[:],
                        in_=m_d[128 * t : 128 * (t + 1), HALF * h : HALF * (h + 1)],
                    )
                    nc.vector.tensor_tensor(
                        out=tprod[:, HALF * h : HALF * (h + 1)],
                        in0=c_t[:],
                        in1=m_t[:],
                        op=OP.mult,
                    )
                    nc.scalar.activation(
                        out=c_t[:],
                        in_=tprod[:, HALF * h : HALF * (h + 1)],
                        func=AF.Identity,
                        accum_out=rs2[:, h : h + 1],
                    )
                # rcpn = -1 / (rowsum + eps)
                rsum = sb.tile([128, 1], F32, tag="rsum", bufs=2, name="rsum")
                nc.vector.tensor_tensor(
                    out=rsum[:], in0=rs2[:, 0:1], in1=rs2[:, 1:2], op=OP.add
                )
                rsneg = sb.tile([128, 1], F32, tag="rsneg", bufs=2, name="rsneg")
                nc.vector.tensor_scalar(
                    rsneg[:], rsum[:], EPS, -1.0, OP.add, OP.mult
                )
                rcpn = sb.tile([128, 1], F32, tag="rcpn", bufs=2, name="rcpn")
                nc.vector.reciprocal(out=rcpn[:], in_=rsneg[:])
                w2 = sb.tile([128, S2], F32, tag="w2", bufs=1, name="w2")
                for h in range(2):
                    e_t = sb.tile([128, HALF], F32, tag="e_t", bufs=2, name="e_t")
                    nc.sync.dma_start(
                        out=e_t[:],
                        in_=e_d[128 * t : 128 * (t + 1), HALF * h : HALF * (h + 1)],
                    )
                    u_t = sb.tile(
                        [128, HALF], F32, tag="h2048", bufs=2, name="u_t"
                    )
                    nc.vector.tensor_scalar(
                        u_t[:],
                        tprod[:, HALF * h : HALF * (h + 1)],
                        rcpn[:, 0:1],
                        None,
                        OP.mult,
                    )
                    # w2 = exc + (-lri); the x2 lateral strength is folded
                    # into the transpose identity (2*I)
                    nc.vector.tensor_tensor(
                        out=w2[:, HALF * h : HALF * (h + 1)],
                        in0=e_t[:],
                        in1=u_t[:],
                        op=OP.add,
                    )
                # transpose 128x128 blocks: WT[k][p(col sub), i(row)] layout
                w2v = w2[:].rearrange("p (i j) -> p i j", j=KC)
                for k in range(KC if do_tr else 0):
                    ps_tr = psp.tile([128, 128], F32, tag="tr", bufs=3, name="ps_tr")
                    nc.tensor.transpose(
                        out=ps_tr[:], in_=w2v[:, :, k], identity=ident[:]
                    )
                    if mode == "f32r":
                        nc.scalar.copy(
                            out=wt_h[:, k, 128 * t : 128 * (t + 1)], in_=ps_tr[:]
                        )
                    else:
                        nc.scalar.copy(
                            out=wt_h[:, k, 128 * t : 128 * (t + 1)], in_=ps_tr[:]
                        )
                        nc.vector.tensor_tensor(
                            out=wt_l[:, k, 128 * t : 128 * (t + 1)],
                            in0=ps_tr[:],
                            in1=wt_h[:, k, 128 * t : 128 * (t + 1)],
                            op=OP.subtract,
                        )

            # ---- afferent - thr as a [1, 512] free-axis vector ----
            aff_b = dr.tile([RT, 128], F32)
            nc.sync.dma_start(out=aff_b[:].transpose([1, 0]), in_=aff_col[:])
            aff_f = sb.tile([1, RP], F32, tag="v512b", bufs=1, name="aff_f")
            nc.sync.dma_start(
                out=aff_f[:], in_=aff_b[:].rearrange("t p -> (t p)").unsqueeze(0)
            )
            thr_f = sb.tile([1, RP], F32, tag="v512a", bufs=1, name="thr_f")
            nc.sync.dma_start(out=thr_f[:], in_=thr_d[:])
            aff2_f = sb.tile([1, RP], F32, name="aff2_f")
            nc.vector.tensor_tensor(
                out=aff2_f[:], in0=aff_f[:], in1=thr_f[:], op=OP.subtract
            )

            # ---- iteration 1: r = relu(tanh(aff2_f)) (== tanh(relu(.))) ----
            r_a = sb.tile([1, RP], F32, tag="v512b", bufs=1, name="r_a")
            nc.scalar.activation(out=r_a[:], in_=aff2_f[:], func=AF.Tanh)
            r_s = sb.tile([1, RP], F32, tag="r_s", bufs=2, name="r_s")
            nc.vector.tensor_scalar_max(r_s[:], r_a[:], 0.0)

            dbg_sb = sb.tile([1, RP], F32, name="dbg_sb")
            if do_w and do_tr:
                nc.vector.tensor_copy(out=dbg_sb[:], in_=wt_h[0:1, 0, :])
            elif do_w:
                nc.vector.tensor_copy(out=dbg_sb[:], in_=w2[0:1, 0:RP])
            else:
                nc.vector.tensor_copy(out=dbg_sb[:], in_=aff2_f[:])
            nc.sync.dma_start(out=dbg_d[:], in_=dbg_sb[:])

            for it in range(1, iters):
                cc_in = dr.tile([1, RP], F32, tag="cc_in", bufs=2, name="cc_in")
                cc_out = dr.tile(
                    [128, KC], F32, tag="cc_out", bufs=2, name="cc_out"
                )
                nc.sync.dma_start(out=cc_in[:], in_=r_s[:])
                if use_ag:
                    nc.gpsimd.collective_compute(
                        "AllGather",
                        OP.bypass,
                        replica_groups=groups,
                        ins=[cc_in[:].opt()],
                        outs=[cc_out[:].opt()],
                    )
                else:
                    # debug: local stand-in for the collective (wrong values)
                    nc.sync.dma_start(
                        out=cc_out[:]
                        .rearrange("p k -> (p k)")
                        .unsqueeze(0)[:, 0:RP],
                        in_=cc_in[:],
                    )
                # rT[p, k] = r[32p + k]
                rT = sb.tile([128, KC], F32, tag="rT", bufs=2, name="rT")
                nc.sync.dma_start(out=rT[:], in_=cc_out[:])

                ps_mv = psp.tile([1, RP], F32, tag="mv", bufs=2, name="ps_mv")
                if mode == "f32r":
                    rr = rT[:].bitcast(F32R)
                    ww = wt_h[:].bitcast(F32R)
                    for k in range(KC):
                        nc.tensor.matmul(
                            ps_mv[:],
                            rr[:, k : k + 1],
                            ww[:, k, :],
                            start=(k == 0),
                            stop=(k == KC - 1),
                        )
                else:
                    # scale by the lateral strength here: matvec yields 2*(W@r)
                    rT2 = sb.tile([128, KC], F32, tag="rT2", bufs=2, name="rT2")
                    nc.vector.tensor_scalar_mul(rT2[:], rT[:], 2.0)
                    r_h = sb.tile([128, KC], BF16, tag="r_h", bufs=2, name="r_h")
                    r_l = sb.tile([128, KC], BF16, tag="r_l", bufs=2, name="r_l")
                    nc.vector.tensor_copy(out=r_h[:], in_=rT2[:])
                    nc.vector.tensor_tensor(
                        out=r_l[:], in0=rT2[:], in1=r_h[:], op=OP.subtract
                    )
                    chains = [(r_h, wt_h), (r_l, wt_h), (r_h, wt_l)]
                    n = 0
                    for lt, wt in chains:
                        for k in range(KC):
                            nc.tensor.matmul(
                                ps_mv[:],
                                lt[:, k : k + 1],
                                wt[:, k, :],
                                start=(n == 0),
                                stop=(n == 3 * KC - 1),
                            )
                            n += 1

                z = sb.tile([1, RP], F32, tag="v512a", bufs=1, name="z")
                nc.vector.tensor_tensor(
                    out=z[:], in0=ps_mv[:], in1=aff2_f[:], op=OP.add
                )
                r_a = sb.tile([1, RP], F32, tag="v512b", bufs=1, name="r_a")
                nc.scalar.activation(out=r_a[:], in_=z[:], func=AF.Tanh)
                r_s = sb.tile([1, RP], F32, tag="r_s", bufs=2, name="r_s")
                nc.vector.tensor_scalar_max(r_s[:], r_a[:], 0.0)

                if it < iters - 1 and mode == "bf16x3":
                    # keep the PE HAM window busy through the AllGather gap so
                    # the next iteration's matmuls run at 2.4 GHz, not 1.2
                    ps_dum = psp.tile([1, RP], F32, tag="dum", bufs=1, name="ps_dum")
                    for _ in range(40):
                        nc.tensor.matmul(
                            ps_dum[:],
                            r_h[:, 0:1],
                            wt_h[:, 0, :],
                            start=True,
                            stop=True,
                        )

            nc.sync.dma_start(out=out_d[:], in_=r_s[:])

    nc.compile()
    return nc


_CACHE = {}


def _get_nc(mode=MODE):
    if mode not in _CACHE:
        _CACHE[mode] = build(mode)
    return _CACHE[mode]


def make_in_maps(
    input_crop, afferent_weights, lateral_weights_exc, lateral_correlations, masks, thresholds
):
    A = np.ascontiguousarray(
        afferent_weights.reshape(S2, S2), dtype=np.float32
    )
    E = np.ascontiguousarray(
        lateral_weights_exc.reshape(S2, S2), dtype=np.float32
    )
    C = np.ascontiguousarray(
        lateral_correlations.reshape(S2, S2), dtype=np.float32
    )
    M = np.ascontiguousarray(masks.reshape(S2, S2), dtype=np.float32)
    x = np.ascontiguousarray(
        input_crop.reshape(1, S2), dtype=np.float32
    )
    xb = np.ascontiguousarray(np.broadcast_to(x, (128, S2)))
    thr = np.ascontiguousarray(thresholds.reshape(S2), dtype=np.float32)
    ident = 2.0 * np.eye(128, dtype=np.float32)  # transpose + lateral strength

    in_maps = []
    for c in range(NCORES):
        sl = slice(RP * c, RP * (c + 1))
        in_maps.append(
            {
                "a_shard": A[sl],
                "e_shard": E[sl],
                "c_shard": C[sl],
                "m_shard": M[sl],
                "x_bcast": xb,
                "thr_shard": thr[sl].reshape(1, RP),
                "ident": ident,
            }
        )
    return in_maps


def run(in_maps, mode=MODE, **kwargs):
    nc = _get_nc(mode)
    return run_bass_kernel_spmd(nc, in_maps, core_ids=list(range(NCORES)), **kwargs)


def kernel(
    input_crop,
    afferent_weights,
    lateral_weights_exc,
    lateral_correlations,
    masks,
    thresholds,
):
    in_maps = make_in_maps(
        input_crop,
        afferent_weights,
        lateral_weights_exc,
        lateral_correlations,
        masks,
        thresholds,
    )
    res = run(in_maps)
    shards = [res.results[c]["r_out"].reshape(RP) for c in range(NCORES)]
    out = np.concatenate(shards).reshape(1, 1, 64, 64).astype(np.float32)
    return out
